# revision 1
# baseline (speedup 1.0000x reference)
"""Bilinear interpolation (spatial transformer) Trainium2 kernel.

Strategy (pure data parallel, 8 images per NeuronCore):
  Per image, build J4 in DRAM: J4[y*384+x] = 12 f32 = the 4 bilinear corner
  pixels [I(y,x,:), I(y,x+1,:), I(y+1,x,:), I(y+1,x+1,:)] (clamped at edges;
  out-of-range halves are zero-weighted by the bilinear weights, matching the
  reference's clip arithmetic exactly).
  Per output pixel, compute the affine sample position, floor/clip (exact
  reference arithmetic), the 4 bilinear weights, and a J4 slot index; gather
  48B per pixel via indirect SWDGE DMA (one descriptor per partition per
  instruction); weighted-combine on DVE; store.

Processing tiles: "thirds" of an image = 128 output rows, one row per
partition, 384 pixels along the free dim.
"""
import sys

sys.path.insert(0, "/opt/trn_rl_repo")

import numpy as np

H = 384
W = 384
C = 3
B = 64
NCORES = 8
BPC = B // NCORES          # images per core
NT = 3                     # thirds per image
ROWS_T = 128               # output rows per third
IMG_ELEMS = H * W * C      # 442368
ROW_ELEMS = W * C          # 1152
NSLOT = H * W              # J4 slots per image

_CACHE = {}


def _build_program():
    import concourse.bass as bass
    import concourse.bacc as bacc
    import concourse.mybir as mybir
    from concourse import tile

    f32 = mybir.dt.float32
    i32 = mybir.dt.int32
    AF = mybir.ActivationFunctionType
    OP = mybir.AluOpType

    nc = bacc.Bacc("TRN2", target_bir_lowering=False, debug=False,
                   num_devices=NCORES)

    images = nc.dram_tensor("images", [BPC * IMG_ELEMS + 1280], f32, kind="ExternalInput")
    theta = nc.dram_tensor("theta", [1, BPC * 6], f32, kind="ExternalInput")
    xs_c = nc.dram_tensor("xs_c", [128, W], f32, kind="ExternalInput")
    ys_c = nc.dram_tensor("ys_c", [128, NT], f32, kind="ExternalInput")
    out_d = nc.dram_tensor("out", [BPC, IMG_ELEMS], f32, kind="ExternalOutput")
    j4 = [nc.dram_tensor(f"j4_{m}", [NSLOT, 12], f32) for m in range(BPC)]

    def dram_ap(t, off, layout):
        return bass.AP(t, off, layout)

    # Phase A in its own TileContext: the context-exit barrier guarantees all
    # J4 DRAM writes land before phase B's indirect gathers read them (Tile
    # does not track the DRAM RAW dependency across those instruction kinds).
    with tile.TileContext(nc) as tc:
        with tc.tile_pool(name="jbuild", bufs=2) as jpool:
            # ---------------- phase A: build J4 per (image, third) -------------
            for m in range(BPC):
                base = m * IMG_ELEMS
                for T in range(NT):
                    im0 = jpool.tile([128, ROW_ELEMS + 8], f32, tag="im0")
                    im1 = jpool.tile([128, ROW_ELEMS + 8], f32, tag="im1")
                    # rows 128T+p  (pad +8 reads into the next row / next image;
                    # images tensor is padded so this never faults)
                    off0 = base + T * 128 * ROW_ELEMS
                    nc.sync.dma_start(
                        im0[:],
                        dram_ap(images, off0, [[ROW_ELEMS, 128], [1, ROW_ELEMS + 8]]),
                    )
                    # rows 128T+p+1. For T==2, partition 127 reads "row 384",
                    # i.e. past the image: next image's row 0, or the zero pad
                    # for the last image. Those taps are zero-weighted
                    # (y0==383 implies all four bilinear weights are 0), so
                    # any finite values are fine.
                    nc.sync.dma_start(
                        im1[:],
                        dram_ap(images, off0 + ROW_ELEMS,
                                [[ROW_ELEMS, 128], [1, ROW_ELEMS + 8]]),
                    )

                    j4t = jpool.tile([128, W * 12], f32, tag="j4t")
                    # slot layout per partition: [x: 384][tap: 4][c: 3]
                    o3 = j4t[:].rearrange("p (x t c) -> p x t c", t=4, c=3)

                    def xc3(t_, off):
                        return t_[:, off : off + W * 3].rearrange("p (x c) -> p x c", c=3)

                    # taps: 0=(y,x) 1=(y,x+1) 2=(y+1,x) 3=(y+1,x+1)
                    nc.scalar.copy(o3[:, :, 0, :], xc3(im0, 0))
                    nc.scalar.copy(o3[:, :, 1, :], xc3(im0, 3))
                    nc.vector.tensor_copy(o3[:, :, 2, :], xc3(im1, 0))
                    nc.vector.tensor_copy(o3[:, :, 3, :], xc3(im1, 3))

                    nc.sync.dma_start(
                        dram_ap(j4[m], T * 128 * W * 12, [[W * 12, 128], [1, W * 12]]),
                        j4t[:],
                    )

    with tile.TileContext(nc) as tc:
        with (
            tc.tile_pool(name="consts", bufs=1) as cpool,
            tc.tile_pool(name="work", bufs=2) as wpool,
            tc.tile_pool(name="gather", bufs=2) as gpool,
        ):
            # constants
            xs_t = cpool.tile([128, W], f32, tag="xs")
            nc.sync.dma_start(xs_t[:], xs_c[:])
            ys_t = cpool.tile([128, NT], f32, tag="ys")
            nc.sync.dma_start(ys_t[:], ys_c[:])
            th_row = cpool.tile([1, BPC * 6], f32, tag="throw")
            nc.sync.dma_start(th_row[:], theta[:])
            th = cpool.tile([128, BPC * 6], f32, tag="th")
            nc.gpsimd.partition_broadcast(th[:], th_row[:])

            # ---------------- phase B: per (image, third) sample+gather --------
            for m in range(BPC):
                t00 = th[:, m * 6 + 0 : m * 6 + 1]
                t01 = th[:, m * 6 + 1 : m * 6 + 2]
                t02 = th[:, m * 6 + 2 : m * 6 + 3]
                t10 = th[:, m * 6 + 3 : m * 6 + 4]
                t11 = th[:, m * 6 + 4 : m * 6 + 5]
                t12 = th[:, m * 6 + 5 : m * 6 + 6]
                for T in range(NT):
                    ysT = ys_t[:, T : T + 1]
                    tiny = wpool.tile([128, 2], f32, tag="tiny")
                    # ys*t01, ys*t11
                    nc.vector.tensor_scalar(tiny[:, 0:1], ysT, t01, None, OP.mult)
                    nc.vector.tensor_scalar(tiny[:, 1:2], ysT, t11, None, OP.mult)

                    x = wpool.tile([128, W], f32, tag="x")
                    y = wpool.tile([128, W], f32, tag="y")
                    nc.vector.tensor_scalar(x[:], xs_t[:], t00, None, OP.mult)
                    nc.vector.tensor_scalar(x[:], x[:], tiny[:, 0:1], t02, OP.add, OP.add)
                    nc.vector.tensor_scalar(x[:], x[:], 1.0, float(W) * 0.5, OP.add, OP.mult)
                    nc.vector.tensor_scalar(y[:], xs_t[:], t10, None, OP.mult)
                    nc.vector.tensor_scalar(y[:], y[:], tiny[:, 1:2], t12, OP.add, OP.add)
                    nc.vector.tensor_scalar(y[:], y[:], 1.0, float(H) * 0.5, OP.add, OP.mult)

                    # floors (round-to-nearest-even of v-0.5 == floor except at
                    # exact integers, where bilinear output is unaffected)
                    x0i = wpool.tile([128, W], i32, tag="x0i")
                    y0i = wpool.tile([128, W], i32, tag="y0i")
                    nc.scalar.activation(x0i[:], x[:], AF.Copy, bias=-0.5)
                    nc.scalar.activation(y0i[:], y[:], AF.Copy, bias=-0.5)
                    x0f = wpool.tile([128, W], f32, tag="x0f")
                    y0f = wpool.tile([128, W], f32, tag="y0f")
                    nc.scalar.activation(x0f[:], x0i[:], AF.Copy)
                    nc.scalar.activation(y0f[:], y0i[:], AF.Copy)

                    x0c = wpool.tile([128, W], f32, tag="x0c")
                    x1c = wpool.tile([128, W], f32, tag="x1c")
                    y0c = wpool.tile([128, W], f32, tag="y0c")
                    y1c = wpool.tile([128, W], f32, tag="y1c")
                    nc.vector.tensor_scalar(x0c[:], x0f[:], 0.0, float(W - 1), OP.max, OP.min)
                    nc.vector.tensor_scalar(x1c[:], x0f[:], -1.0, 1.0, OP.max, OP.add)
                    nc.vector.tensor_scalar(x1c[:], x1c[:], float(W - 1), None, OP.min)
                    nc.vector.tensor_scalar(y0c[:], y0f[:], 0.0, float(H - 1), OP.max, OP.min)
                    nc.vector.tensor_scalar(y1c[:], y0f[:], -1.0, 1.0, OP.max, OP.add)
                    nc.vector.tensor_scalar(y1c[:], y1c[:], float(H - 1), None, OP.min)

                    xc = wpool.tile([128, W], f32, tag="xc")
                    yc = wpool.tile([128, W], f32, tag="yc")
                    nc.vector.tensor_scalar(xc[:], x[:], 0.0, float(W - 1), OP.max, OP.min)
                    nc.vector.tensor_scalar(yc[:], y[:], 0.0, float(H - 1), OP.max, OP.min)

                    dxa = wpool.tile([128, W], f32, tag="dxa")
                    dxc = wpool.tile([128, W], f32, tag="dxc")
                    dya = wpool.tile([128, W], f32, tag="dya")
                    dyb = wpool.tile([128, W], f32, tag="dyb")
                    nc.vector.tensor_tensor(dxa[:], x1c[:], xc[:], OP.subtract)
                    nc.vector.tensor_tensor(dxc[:], xc[:], x0c[:], OP.subtract)
                    nc.vector.tensor_tensor(dya[:], y1c[:], yc[:], OP.subtract)
                    nc.vector.tensor_tensor(dyb[:], yc[:], y0c[:], OP.subtract)

                    wa = wpool.tile([128, W], f32, tag="wa")
                    wb = wpool.tile([128, W], f32, tag="wb")
                    wc_ = wpool.tile([128, W], f32, tag="wc")
                    wd = wpool.tile([128, W], f32, tag="wd")
                    nc.vector.tensor_tensor(wa[:], dxa[:], dya[:], OP.mult)
                    nc.vector.tensor_tensor(wb[:], dxa[:], dyb[:], OP.mult)
                    nc.vector.tensor_tensor(wc_[:], dxc[:], dya[:], OP.mult)
                    nc.vector.tensor_tensor(wd[:], dxc[:], dyb[:], OP.mult)

                    vf = wpool.tile([128, W], f32, tag="vf")
                    nc.vector.scalar_tensor_tensor(
                        vf[:], y0c[:], float(W), x0c[:], op0=OP.mult, op1=OP.add
                    )
                    vi = wpool.tile([128, W], i32, tag="vi")
                    nc.vector.tensor_copy(vi[:], vf[:])

                    g = gpool.tile([128, W * 12], f32, tag="g")
                    g3 = g[:].rearrange("p (k s) -> p k s", s=12)
                    for k in range(W):
                        nc.gpsimd.indirect_dma_start(
                            out=g[:, k * 12 : (k + 1) * 12],
                            out_offset=None,
                            in_=j4[m][:],
                            in_offset=bass.IndirectOffsetOnAxis(ap=vi[:, k : k + 1], axis=0),
                        )

                    def wbc(t_):
                        return t_[:].rearrange("p (k one) -> p k one", one=1).to_broadcast([128, W, 3])

                    acc = wpool.tile([128, W * 3], f32, tag="acc")
                    tmp = wpool.tile([128, W * 3], f32, tag="tmp")
                    a3 = acc[:].rearrange("p (k c) -> p k c", c=3)
                    t3 = tmp[:].rearrange("p (k c) -> p k c", c=3)
                    g4 = g[:].rearrange("p (k t c) -> p k t c", t=4, c=3)
                    nc.vector.tensor_tensor(a3[:], g4[:, :, 0, :], wbc(wa), OP.mult)
                    nc.vector.tensor_tensor(t3[:], g4[:, :, 1, :], wbc(wc_), OP.mult)
                    nc.vector.tensor_tensor(a3[:], a3[:], t3[:], OP.add)
                    nc.vector.tensor_tensor(t3[:], g4[:, :, 2, :], wbc(wb), OP.mult)
                    nc.vector.tensor_tensor(a3[:], a3[:], t3[:], OP.add)
                    nc.vector.tensor_tensor(t3[:], g4[:, :, 3, :], wbc(wd), OP.mult)
                    nc.vector.tensor_tensor(a3[:], a3[:], t3[:], OP.add)

                    nc.sync.dma_start(
                        dram_ap(out_d, m * IMG_ELEMS + T * 128 * ROW_ELEMS,
                                [[ROW_ELEMS, 128], [1, ROW_ELEMS]]),
                        acc[:],
                    )

    nc.compile()
    return nc


class _Runner:
    def __init__(self, nc, n_cores):
        import jax
        from jax.sharding import Mesh, PartitionSpec
        from jax.experimental.shard_map import shard_map
        import concourse.mybir as mybir
        from concourse.bass2jax import (
            _bass_exec_p, partition_id_tensor, install_neuronx_cc_hook,
        )

        install_neuronx_cc_hook()
        self.jax = jax
        self.n_cores = n_cores
        partition_name = nc.partition_id_tensor.name if nc.partition_id_tensor else None
        in_names, out_names, out_avals, zero_outs = [], [], [], []
        for alloc in nc.m.functions[0].allocations:
            if not isinstance(alloc, mybir.MemoryLocationSet):
                continue
            name = alloc.memorylocations[0].name
            if alloc.kind == "ExternalInput":
                if name != partition_name:
                    in_names.append(name)
            elif alloc.kind == "ExternalOutput":
                shape = tuple(alloc.tensor_shape)
                dtype = mybir.dt.np(alloc.dtype)
                out_avals.append(jax.core.ShapedArray(shape, dtype))
                out_names.append(name)
                zero_outs.append(np.zeros(shape, dtype))
        self.in_names = list(in_names)
        self.out_names = out_names
        self.zero_outs = zero_outs
        n_params = len(in_names)
        n_outs = len(out_names)
        all_in_names = in_names + out_names
        if partition_name is not None:
            all_in_names.append(partition_name)

        def _body(*args):
            operands = list(args)
            if partition_name is not None:
                operands.append(partition_id_tensor())
            outs = _bass_exec_p.bind(
                *operands,
                out_avals=tuple(out_avals),
                in_names=tuple(all_in_names),
                out_names=tuple(out_names),
                lowering_input_output_aliases=(),
                sim_require_finite=False,
                sim_require_nnan=False,
                nc=nc,
            )
            return tuple(outs)

        devices = jax.devices()[:n_cores]
        self.mesh = Mesh(np.asarray(devices), ("core",))
        in_specs = (PartitionSpec("core"),) * (n_params + n_outs)
        out_specs = (PartitionSpec("core"),) * n_outs
        self.fn = jax.jit(
            shard_map(_body, mesh=self.mesh, in_specs=in_specs,
                      out_specs=out_specs, check_rep=False),
            keep_unused=True,
        )

    def run(self, in_maps):
        from jax.sharding import NamedSharding, PartitionSpec
        sharding = NamedSharding(self.mesh, PartitionSpec("core"))
        concat = [
            np.concatenate([np.asarray(m[name]) for m in in_maps], axis=0)
            for name in self.in_names
        ]
        concat += [
            np.zeros((self.n_cores * z.shape[0], *z.shape[1:]), z.dtype)
            for z in self.zero_outs
        ]
        args = [self.jax.device_put(a, sharding) for a in concat]
        outs = self.fn(*args)
        self.jax.block_until_ready(outs)
        res = []
        for c in range(self.n_cores):
            d = {}
            for i, name in enumerate(self.out_names):
                a = np.asarray(outs[i])
                per_core = (self.n_cores, a.shape[0] // self.n_cores) + a.shape[1:]
                d[name] = a.reshape(per_core)[c]
            res.append(d)
        return res


def _get_runner():
    if "runner" not in _CACHE:
        nc = _build_program()
        _CACHE["runner"] = _Runner(nc, NCORES)
    return _CACHE["runner"]


def _host_constants():
    import jax.numpy as jnp

    xs = np.asarray(jnp.linspace(-1.0, 1.0, W, dtype=jnp.float32))
    ys = np.asarray(jnp.linspace(-1.0, 1.0, H, dtype=jnp.float32))
    xs_c = np.tile(xs[None, :], (128, 1)).astype(np.float32)
    # ys_c[p, T] = ys[128*T + p]
    ys_c = ys.reshape(NT, 128).T.copy().astype(np.float32)
    return xs_c, ys_c


def kernel(images, theta):
    images = np.ascontiguousarray(images, dtype=np.float32)
    theta = np.ascontiguousarray(theta, dtype=np.float32)
    assert images.shape == (B, H, W, C) and theta.shape == (B, 2, 3)
    runner = _get_runner()
    xs_c, ys_c = _host_constants()
    in_maps = []
    for c in range(NCORES):
        imgs = images[c * BPC : (c + 1) * BPC].reshape(-1)
        imgs = np.concatenate([imgs, np.zeros(1280, np.float32)])
        th = theta[c * BPC : (c + 1) * BPC].reshape(1, BPC * 6)
        in_maps.append({"images": imgs, "theta": th, "xs_c": xs_c, "ys_c": ys_c})
    res = runner.run(in_maps)
    out = np.empty((B, H, W, C), np.float32)
    for c in range(NCORES):
        out[c * BPC : (c + 1) * BPC] = res[c]["out"].reshape(BPC, H, W, C)
    return out



# revision 20
# speedup vs baseline: 1.0365x; 1.0365x over previous
"""Bilinear interpolation (spatial transformer) Trainium2 kernel.

Strategy (data parallel, 8 images per NeuronCore, theta-specialized):

Phase A (per image): build a DRAM table of window slots. Slot (y, k) holds
rows {y, y+1} x cols [5k, 5k+6) x 3 channels in bf16, layout [row][col][ch]
(36 bf16 used of a 128-bf16 / 256B element). 384*77 = 29568 slots per image,
so slot ids fit int16 -- required by dma_gather. Only y-tiles actually
sampled (host-computed from theta) are built.

Phase B (per output third = 128 rows x 384 cols): compute the affine sample
position and the reference's exact floor/clip arithmetic on device; slot id
F = y0*77 + floor(x0/5) (floors via the f32 magic-number trick, 1.5*2^23).
One 256B dma_gather element per output pixel, 1024 indices per instruction
(the SWDGE descriptor-ring carveout limit). The index list is wrapped
[16, n/16] replicated across the 8 GPSIMD core groups; per-q selection
matmuls on the otherwise-idle tensor engine produce that layout directly.
Horizontal selection in the 6-wide window is the stencil
u[b] = wx0*[b==rx] + wx1*[b==rx+1]; vertical uses the reference weights.

Theta specialization: out-of-range pixels are exactly 0 (the reference's
clip arithmetic zero-weights them), and the active region per output row is
an interval, convex over each third. The host computes a conservative
active column range per (image, third); gathers/compute outside it are
skipped statically and zeros are written instead. Because activity differs
per image, each core gets its own specialized program, with images assigned
to cores balancing total active work. Programs are rebuilt if theta changes
(cached on theta bytes), so the kernel stays correct for any input.
"""
import sys

sys.path.insert(0, "/opt/trn_rl_repo")

import numpy as np
import ml_dtypes

H = 384
W = 384
C = 3
B = 64
NCORES = 8
BPC = B // NCORES          # images per core
NT = 3                     # thirds per image
IMG_ELEMS = H * W * C      # 442368
ROW_ELEMS = W * C          # 1152
PAD = 4096                 # padding after the last image (overhang reads)

S = 5                      # slot x-stride (pixels)
EW = 6                     # slot window width (pixels)
NK = 77                    # slots per row (k = floor(x0/5) in 0..76)
NSLOT = H * NK             # 29568 slots per image (< 32768: int16 indices)
ESZ = 128                  # bf16 elems per slot element (256B)

GC = 8                     # output columns per gather (1024 descriptors)
JC = 64                    # output columns per selection chunk
NCHUNK = W // JC           # 6

_CACHE = {}


def _activity(theta_all):
    """Host-side conservative activity analysis per image (float64, +/-2px
    margins). Returns per image: thirds{T: (cs_lo, cs_hi) | None} at GC
    granularity, build (list of y-tiles to build), cost (active gathers)."""
    xs = np.linspace(-1.0, 1.0, W)
    ys = np.linspace(-1.0, 1.0, H)
    xn, yn = np.meshgrid(xs, ys)
    out = []
    for m in range(theta_all.shape[0]):
        t = theta_all[m].astype(np.float64)
        x = ((t[0, 0] * xn + t[0, 1] * yn + t[0, 2]) + 1.0) * (W * 0.5)
        y = ((t[1, 0] * xn + t[1, 1] * yn + t[1, 2]) + 1.0) * (H * 0.5)
        act = (x > -2) & (x < W + 1) & (y > -2) & (y < H + 1)
        thirds = {}
        ybounds = []
        cost = 0
        for T in range(NT):
            a = act[128 * T : 128 * T + 128]
            ja = a.any(axis=0)
            if not ja.any():
                thirds[T] = None
                continue
            cs = ja.reshape(W // GC, GC).any(axis=1)
            lo = int(np.argmax(cs))
            hi = (W // GC) - int(np.argmax(cs[::-1]))
            thirds[T] = (lo, hi)
            cost += hi - lo
            # slot rows referenced by gathers = clip(floor(y), 0, 383) over
            # the WHOLE issued-chunk region (inactive pixels there carry
            # clipped slot ids too; their windows are gathered then
            # zero-weighted, so the rows must exist in the table)
            yr = y[128 * T : 128 * T + 128, GC * lo : GC * hi]
            ylo_r = min(H - 1, max(0, int(np.floor(yr.min())) - 2))
            yhi_r = min(H - 1, max(0, int(np.floor(yr.max())) + 2))
            ybounds.append((ylo_r, yhi_r))
        build = set()
        for (ylo, yhi) in ybounds:
            for Ty in range(NT):
                if ylo <= 128 * Ty + 127 and yhi >= 128 * Ty:
                    build.add(Ty)
        out.append({"thirds": thirds, "build": sorted(build), "cost": cost})
    return out


def _assign(acts):
    """Balance images over cores by active-gather cost; 8 images per core."""
    order = sorted(range(len(acts)), key=lambda m: -acts[m]["cost"])
    loads = [0.0] * NCORES
    slots = [[] for _ in range(NCORES)]
    for m in order:
        c = min((c for c in range(NCORES) if len(slots[c]) < BPC),
                key=lambda c: loads[c])
        slots[c].append(m)
        loads[c] += acts[m]["cost"] + 6  # small constant per image (phase A)
    return slots


def _build_program(theta8, acts8):
    """Build one core's program, specialized to its 8 images' activity."""
    import concourse.bass as bass
    import concourse.bacc as bacc
    import concourse.mybir as mybir
    from concourse import tile

    f32 = mybir.dt.float32
    bf16 = mybir.dt.bfloat16
    i16 = mybir.dt.int16
    OP = mybir.AluOpType

    nc = bacc.Bacc("TRN2", target_bir_lowering=False, debug=False,
                   num_devices=1)

    images = nc.dram_tensor("images", [BPC * IMG_ELEMS + PAD], f32,
                            kind="ExternalInput")
    theta = nc.dram_tensor("theta", [1, BPC * 6], f32, kind="ExternalInput")
    xs_c = nc.dram_tensor("xs_c", [128, W], f32, kind="ExternalInput")
    ys_c = nc.dram_tensor("ys_c", [128, NT], f32, kind="ExternalInput")
    mq_c = nc.dram_tensor("mq_c", [128, 8 * 128], f32, kind="ExternalInput")
    iota_c = nc.dram_tensor("iota_c", [128, W * EW], bf16, kind="ExternalInput")
    out_d = nc.dram_tensor("out", [BPC, IMG_ELEMS], f32, kind="ExternalOutput")
    tabs = [nc.dram_tensor(f"tab_{m}", [NSLOT, ESZ], bf16) for m in range(BPC)]

    def dram_ap(t, off, layout):
        return bass.AP(t, off, layout)

    # ---------------- phase A: build window-slot tables --------------------
    with tile.TileContext(nc) as tc:
        with (
            tc.tile_pool(name="abuild", bufs=2) as apool,
            tc.tile_pool(name="atb", bufs=1) as tpool,
        ):
            tbs = [tpool.tile([128, NK * ESZ], bf16, tag=f"tb{i}",
                              name=f"tb{i}")
                   for i in range(2)]
            nc.vector.memset(tbs[0][:], 0.0)
            nc.vector.memset(tbs[1][:], 0.0)
            nbuilt = 0
            for m in range(BPC):
                base = m * IMG_ELEMS
                for T in acts8[m]["build"]:
                    im0 = apool.tile([128, ROW_ELEMS + 24], f32, tag="im0")
                    im1 = apool.tile([128, ROW_ELEMS + 24], f32, tag="im1")
                    off0 = base + T * 128 * ROW_ELEMS
                    nc.sync.dma_start(
                        im0[:],
                        dram_ap(images, off0,
                                [[ROW_ELEMS, 128], [1, ROW_ELEMS + 24]]),
                    )
                    # rows +1; for T==2 partition 127 reads row 384 = next
                    # image / zero pad: those taps are zero-weighted.
                    nc.sync.dma_start(
                        im1[:],
                        dram_ap(images, off0 + ROW_ELEMS,
                                [[ROW_ELEMS, 128], [1, ROW_ELEMS + 24]]),
                    )
                    tb = tbs[nbuilt % 2]
                    nbuilt += 1
                    tb3 = tb[:].rearrange("p (k w) -> p k w", w=ESZ)

                    def win15(t_):
                        a = t_[:, 0 : NK * 15].rearrange(
                            "p (k x) -> p k x", x=15)
                        b_ = t_[:, 15 : 15 + NK * 15].rearrange(
                            "p (k x) -> p k x", x=15)
                        return a, b_

                    a0, b0 = win15(im0)
                    a1, b1 = win15(im1)
                    nc.scalar.copy(tb3[:, :, 0:15], a0)
                    nc.scalar.copy(tb3[:, :, 15:18], b0[:, :, 0:3])
                    nc.vector.tensor_copy(tb3[:, :, 18:33], a1)
                    nc.vector.tensor_copy(tb3[:, :, 33:36], b1[:, :, 0:3])
                    nc.sync.dma_start(
                        dram_ap(tabs[m], T * 128 * NK * ESZ,
                                [[NK * ESZ, 128], [1, NK * ESZ]]),
                        tb[:],
                    )

    # ---------------- phase B: sample + gather + combine -------------------
    with tile.TileContext(nc) as tc:
        with (
            tc.tile_pool(name="consts", bufs=1) as cpool,
            tc.tile_pool(name="work", bufs=2) as wpool,
            tc.tile_pool(name="idx", bufs=2) as ipool,
            tc.tile_pool(name="sel", bufs=2) as spool,
            tc.tile_pool(name="ps", bufs=2, space="PSUM") as ppool,
        ):
            xs_t = cpool.tile([128, W], f32, tag="xs")
            nc.sync.dma_start(xs_t[:], xs_c[:])
            ys_t = cpool.tile([128, NT], f32, tag="ys")
            nc.sync.dma_start(ys_t[:], ys_c[:])
            mq_t = cpool.tile([128, 8 * 128], f32, tag="mq")
            nc.sync.dma_start(mq_t[:], mq_c[:])
            iota_t = cpool.tile([128, W * EW], bf16, tag="iota")
            nc.sync.dma_start(iota_t[:], iota_c[:])
            th_row = cpool.tile([1, BPC * 6], f32, tag="throw")
            nc.sync.dma_start(th_row[:], theta[:])
            th = cpool.tile([128, BPC * 6], f32, tag="th")
            nc.gpsimd.partition_broadcast(th[:], th_row[:])
            zacc = cpool.tile([128, ROW_ELEMS], f32, tag="zacc")
            nc.vector.memset(zacc[:], 0.0)
            gts = [cpool.tile([128, JC * ESZ], bf16, tag=f"g{i}",
                              name=f"g{i}")
                   for i in range(2)]
            nc.vector.memset(gts[0][:], 0.0)
            nc.vector.memset(gts[1][:], 0.0)

            i6b = iota_t[:].rearrange("p (j b) -> p j b", b=EW)
            MAGIC = 12582912.0  # 1.5*2^23: integer-ULP zone for |v| < 2^22
            ngat = 0

            for m in range(BPC):
                t00 = th[:, m * 6 + 0 : m * 6 + 1]
                t01 = th[:, m * 6 + 1 : m * 6 + 2]
                t02 = th[:, m * 6 + 2 : m * 6 + 3]
                t10 = th[:, m * 6 + 3 : m * 6 + 4]
                t11 = th[:, m * 6 + 4 : m * 6 + 5]
                t12 = th[:, m * 6 + 5 : m * 6 + 6]
                for T in range(NT):
                    orng = acts8[m]["thirds"][T]
                    obase = m * IMG_ELEMS + T * 128 * ROW_ELEMS
                    if orng is None:
                        nc.sync.dma_start(
                            dram_ap(out_d, obase,
                                    [[ROW_ELEMS, 128], [1, ROW_ELEMS]]),
                            zacc[:],
                        )
                        continue
                    cs_lo, cs_hi = orng
                    cc_lo, cc_hi = cs_lo // 8, -(-cs_hi // 8)
                    jlo, jhi = JC * cc_lo, JC * cc_hi
                    jw = jhi - jlo
                    jsl = slice(jlo, jhi)

                    ysT = ys_t[:, T : T + 1]
                    sc = wpool.tile([128, 8], f32, tag="sc")
                    nc.vector.tensor_scalar(sc[:, 0:1], t00, float(W) * 0.5, None, OP.mult)
                    nc.vector.tensor_scalar(sc[:, 1:2], ysT, t01, t02, OP.mult, OP.add)
                    nc.vector.tensor_scalar(sc[:, 2:3], sc[:, 1:2], 1.0, float(W) * 0.5, OP.add, OP.mult)
                    nc.vector.tensor_scalar(sc[:, 3:4], t10, float(H) * 0.5, None, OP.mult)
                    nc.vector.tensor_scalar(sc[:, 4:5], ysT, t11, t12, OP.mult, OP.add)
                    nc.vector.tensor_scalar(sc[:, 5:6], sc[:, 4:5], 1.0, float(H) * 0.5, OP.add, OP.mult)

                    x = wpool.tile([128, W], f32, tag="x")
                    y = wpool.tile([128, W], f32, tag="y")
                    nc.vector.tensor_scalar(x[:, jsl], xs_t[:, jsl], sc[:, 0:1],
                                            sc[:, 2:3], OP.mult, OP.add)
                    nc.vector.tensor_scalar(y[:, jsl], xs_t[:, jsl], sc[:, 3:4],
                                            sc[:, 5:6], OP.mult, OP.add)

                    x0f = wpool.tile([128, W], f32, tag="x0f")
                    y0f = wpool.tile([128, W], f32, tag="y0f")
                    nc.vector.tensor_scalar(x0f[:, jsl], x[:, jsl], -0.5, MAGIC, OP.add, OP.add)
                    nc.vector.tensor_scalar(x0f[:, jsl], x0f[:, jsl], -MAGIC, None, OP.add)
                    nc.vector.tensor_scalar(y0f[:, jsl], y[:, jsl], -0.5, MAGIC, OP.add, OP.add)
                    nc.vector.tensor_scalar(y0f[:, jsl], y0f[:, jsl], -MAGIC, None, OP.add)

                    x0c = wpool.tile([128, W], f32, tag="x0c")
                    x1c = wpool.tile([128, W], f32, tag="x1c")
                    y0c = wpool.tile([128, W], f32, tag="y0c")
                    y1c = wpool.tile([128, W], f32, tag="y1c")
                    nc.vector.tensor_scalar(x0c[:, jsl], x0f[:, jsl], 0.0, float(W - 1), OP.max, OP.min)
                    nc.vector.tensor_scalar(x1c[:, jsl], x0f[:, jsl], -1.0, 1.0, OP.max, OP.add)
                    nc.vector.tensor_scalar(x1c[:, jsl], x1c[:, jsl], float(W - 1), None, OP.min)
                    nc.vector.tensor_scalar(y0c[:, jsl], y0f[:, jsl], 0.0, float(H - 1), OP.max, OP.min)
                    nc.vector.tensor_scalar(y1c[:, jsl], y0f[:, jsl], -1.0, 1.0, OP.max, OP.add)
                    nc.vector.tensor_scalar(y1c[:, jsl], y1c[:, jsl], float(H - 1), None, OP.min)

                    xc = wpool.tile([128, W], f32, tag="xc")
                    yc = wpool.tile([128, W], f32, tag="yc")
                    nc.vector.tensor_scalar(xc[:, jsl], x[:, jsl], 0.0, float(W - 1), OP.max, OP.min)
                    nc.vector.tensor_scalar(yc[:, jsl], y[:, jsl], 0.0, float(H - 1), OP.max, OP.min)

                    wx0 = wpool.tile([128, W], f32, tag="wx0")
                    wx1 = wpool.tile([128, W], f32, tag="wx1")
                    wy0 = wpool.tile([128, W], f32, tag="wy0")
                    wy1 = wpool.tile([128, W], f32, tag="wy1")
                    nc.vector.tensor_tensor(wx0[:, jsl], x1c[:, jsl], xc[:, jsl], OP.subtract)
                    nc.vector.tensor_tensor(wx1[:, jsl], xc[:, jsl], x0c[:, jsl], OP.subtract)
                    nc.vector.tensor_tensor(wy0[:, jsl], y1c[:, jsl], yc[:, jsl], OP.subtract)
                    nc.vector.tensor_tensor(wy1[:, jsl], yc[:, jsl], y0c[:, jsl], OP.subtract)

                    # k = floor(x0c/5): v=0.2f*x0c is within 1e-5 of the grid
                    # {m+0.2i}, so bias -0.49 keeps rne(v-0.49)==m tie-free
                    kf = wpool.tile([128, W], f32, tag="kf")
                    nc.vector.tensor_scalar(
                        kf[:, jsl], x0c[:, jsl], float(np.float32(1.0 / S)),
                        -0.49, OP.mult, OP.add)
                    nc.vector.tensor_scalar(kf[:, jsl], kf[:, jsl], MAGIC, None, OP.add)
                    nc.vector.tensor_scalar(kf[:, jsl], kf[:, jsl], -MAGIC, None, OP.add)
                    rx = wpool.tile([128, W], f32, tag="rx")
                    rxp = wpool.tile([128, W], f32, tag="rxp")
                    nc.vector.scalar_tensor_tensor(
                        rx[:, jsl], kf[:, jsl], float(-S), x0c[:, jsl],
                        op0=OP.mult, op1=OP.add)
                    nc.vector.tensor_scalar(rxp[:, jsl], rx[:, jsl], 1.0, None, OP.add)
                    fsl = wpool.tile([128, W], f32, tag="fsl")
                    nc.vector.scalar_tensor_tensor(
                        fsl[:, jsl], y0c[:, jsl], float(NK), kf[:, jsl],
                        op0=OP.mult, op1=OP.add)

                    rx16 = wpool.tile([128, W], bf16, tag="rx16")
                    rxp16 = wpool.tile([128, W], bf16, tag="rxp16")
                    wx016 = wpool.tile([128, W], bf16, tag="wx016")
                    wx116 = wpool.tile([128, W], bf16, tag="wx116")
                    wy016 = wpool.tile([128, W], bf16, tag="wy016")
                    wy116 = wpool.tile([128, W], bf16, tag="wy116")
                    nc.scalar.copy(rx16[:, jsl], rx[:, jsl])
                    nc.scalar.copy(rxp16[:, jsl], rxp[:, jsl])
                    nc.scalar.copy(wx016[:, jsl], wx0[:, jsl])
                    nc.scalar.copy(wx116[:, jsl], wx1[:, jsl])
                    nc.scalar.copy(wy016[:, jsl], wy0[:, jsl])
                    nc.scalar.copy(wy116[:, jsl], wy1[:, jsl])

                    def jb(t_):
                        return t_[:, jsl].rearrange("p (j o) -> p j o", o=1) \
                                         .to_broadcast([128, jw, EW])

                    u = wpool.tile([128, W * EW], bf16, tag="u")
                    ut = wpool.tile([128, W * EW], bf16, tag="ut")
                    u3 = u[:].rearrange("p (j b) -> p j b", b=EW)[:, jsl, :]
                    ut3 = ut[:].rearrange("p (j b) -> p j b", b=EW)[:, jsl, :]
                    i6s = i6b[:, jsl, :]
                    nc.vector.tensor_tensor(u3, i6s, jb(rx16), OP.is_equal)
                    nc.vector.tensor_tensor(u3, u3, jb(wx016), OP.mult)
                    nc.vector.tensor_tensor(ut3, i6s, jb(rxp16), OP.is_equal)
                    nc.vector.tensor_tensor(ut3, ut3, jb(wx116), OP.mult)
                    nc.vector.tensor_tensor(u3, u3, ut3, OP.add)

                    # wrapped+replicated int16 indices:
                    # idx16[P, 8j+q] = fsl[16q + P%16, j]
                    idx16 = ipool.tile([128, W * 8], i16, tag="idx16")
                    idxq = idx16[:].rearrange("p (j q) -> p j q", q=8)
                    for q in range(8):
                        ps = ppool.tile([128, W], f32, tag="ps")
                        nc.tensor.matmul(
                            out=ps[:, 0:jw],
                            lhsT=mq_t[:, q * 128 : (q + 1) * 128],
                            rhs=fsl[:, jsl],
                            start=True,
                            stop=True,
                        )
                        nc.vector.tensor_copy(idxq[:, jsl, q], ps[:, 0:jw])

                    acc = wpool.tile([128, ROW_ELEMS], f32, tag="acc")
                    for cc in range(cc_lo, cc_hi):
                        g = gts[ngat % 2]
                        ngat += 1
                        g3 = g[:].rearrange("p (j w) -> p j w", w=ESZ)
                        for s_ in range(8):
                            cs = cc * 8 + s_
                            if cs < cs_lo or cs >= cs_hi:
                                continue
                            nc.gpsimd.dma_gather(
                                g[:, GC * ESZ * s_ : GC * ESZ * (s_ + 1)]
                                    .rearrange("p (b e) -> p b e", e=ESZ),
                                tabs[m][:],
                                idx16[:, GC * 8 * cs : GC * 8 * (cs + 1)],
                                GC * 128,
                                GC * 128,
                                ESZ,
                            )
                        js = slice(JC * cc, JC * (cc + 1))

                        def cb(t_):
                            return t_[:, js].rearrange("p (j o) -> p j o", o=1) \
                                            .to_broadcast([128, JC, C])

                        u4 = u[:].rearrange("p (j b) -> p j b", b=EW)
                        a0 = spool.tile([128, JC * C], bf16, tag="a0")
                        a1 = spool.tile([128, JC * C], bf16, tag="a1")
                        tt_ = spool.tile([128, JC * C], bf16, tag="tt")
                        t3f = spool.tile([128, JC * C], f32, tag="t3f")
                        a03 = a0[:].rearrange("p (j c) -> p j c", c=C)
                        a13 = a1[:].rearrange("p (j c) -> p j c", c=C)
                        t3 = tt_[:].rearrange("p (j c) -> p j c", c=C)
                        t3f3 = t3f[:].rearrange("p (j c) -> p j c", c=C)

                        def ub(b):
                            return u4[:, js, b : b + 1].to_broadcast([128, JC, C])

                        for r, ar in ((0, a03), (1, a13)):
                            base_w = r * EW * C
                            nc.vector.tensor_tensor(
                                ar, g3[:, :, base_w : base_w + C], ub(0), OP.mult)
                            for b_ in range(1, EW):
                                nc.vector.tensor_tensor(
                                    t3, g3[:, :, base_w + b_ * C : base_w + (b_ + 1) * C],
                                    ub(b_), OP.mult)
                                nc.vector.tensor_tensor(ar, ar, t3, OP.add)

                        a3 = acc[:, JC * C * cc : JC * C * (cc + 1)] \
                            .rearrange("p (j c) -> p j c", c=C)
                        nc.vector.tensor_tensor(a3, a03, cb(wy016), OP.mult)
                        nc.vector.tensor_tensor(t3f3, a13, cb(wy116), OP.mult)
                        nc.vector.tensor_tensor(a3, a3, t3f3, OP.add)

                    # output: zeros left flank | acc active span | zeros right
                    if cc_lo > 0:
                        nc.sync.dma_start(
                            dram_ap(out_d, obase,
                                    [[ROW_ELEMS, 128], [1, JC * C * cc_lo]]),
                            zacc[:, 0 : JC * C * cc_lo],
                        )
                    nc.sync.dma_start(
                        dram_ap(out_d, obase + JC * C * cc_lo,
                                [[ROW_ELEMS, 128], [1, JC * C * (cc_hi - cc_lo)]]),
                        acc[:, JC * C * cc_lo : JC * C * cc_hi],
                    )
                    if cc_hi < NCHUNK:
                        nc.sync.dma_start(
                            dram_ap(out_d, obase + JC * C * cc_hi,
                                    [[ROW_ELEMS, 128],
                                     [1, ROW_ELEMS - JC * C * cc_hi]]),
                            zacc[:, JC * C * cc_hi : ROW_ELEMS],
                        )

    nc.compile()
    return nc


class _Runner:
    def __init__(self, nc, device):
        import jax
        from jax.sharding import Mesh, PartitionSpec
        from jax.experimental.shard_map import shard_map
        import concourse.mybir as mybir
        from concourse.bass2jax import (
            _bass_exec_p, partition_id_tensor, install_neuronx_cc_hook,
        )

        install_neuronx_cc_hook()
        self.jax = jax
        partition_name = nc.partition_id_tensor.name if nc.partition_id_tensor else None
        in_names, out_names, out_avals, zero_outs = [], [], [], []
        for alloc in nc.m.functions[0].allocations:
            if not isinstance(alloc, mybir.MemoryLocationSet):
                continue
            name = alloc.memorylocations[0].name
            if alloc.kind == "ExternalInput":
                if name != partition_name:
                    in_names.append(name)
            elif alloc.kind == "ExternalOutput":
                shape = tuple(alloc.tensor_shape)
                dtype = mybir.dt.np(alloc.dtype)
                out_avals.append(jax.core.ShapedArray(shape, dtype))
                out_names.append(name)
                zero_outs.append(np.zeros(shape, dtype))
        self.in_names = list(in_names)
        self.out_names = out_names
        self.zero_outs = zero_outs
        n_params = len(in_names)
        n_outs = len(out_names)
        all_in_names = in_names + out_names
        if partition_name is not None:
            all_in_names.append(partition_name)

        def _body(*args):
            operands = list(args)
            if partition_name is not None:
                operands.append(partition_id_tensor())
            outs = _bass_exec_p.bind(
                *operands,
                out_avals=tuple(out_avals),
                in_names=tuple(all_in_names),
                out_names=tuple(out_names),
                lowering_input_output_aliases=(),
                sim_require_finite=False,
                sim_require_nnan=False,
                nc=nc,
            )
            return tuple(outs)

        self.mesh = Mesh(np.asarray([device]), ("core",))
        in_specs = (PartitionSpec("core"),) * (n_params + n_outs)
        out_specs = (PartitionSpec("core"),) * n_outs
        self.fn = jax.jit(
            shard_map(_body, mesh=self.mesh, in_specs=in_specs,
                      out_specs=out_specs, check_rep=False),
            keep_unused=True,
        )

    def make_args(self, in_map):
        from jax.sharding import NamedSharding, PartitionSpec
        sharding = NamedSharding(self.mesh, PartitionSpec("core"))
        arrs = [np.asarray(in_map[name]) for name in self.in_names]
        arrs += [z for z in self.zero_outs]
        return [self.jax.device_put(a, sharding) for a in arrs]

    def run(self, in_map):
        outs = self.fn(*self.make_args(in_map))
        self.jax.block_until_ready(outs)
        return {name: np.asarray(outs[i])
                for i, name in enumerate(self.out_names)}


def _host_constants():
    import jax.numpy as jnp

    xs = np.asarray(jnp.linspace(-1.0, 1.0, W, dtype=jnp.float32))
    ys = np.asarray(jnp.linspace(-1.0, 1.0, H, dtype=jnp.float32))
    xs_c = np.tile(xs[None, :], (128, 1)).astype(np.float32)
    ys_c = ys.reshape(NT, 128).T.copy().astype(np.float32)
    mq = np.zeros((128, 8 * 128), np.float32)
    for q in range(8):
        for P in range(128):
            mq[16 * q + (P % 16), q * 128 + P] = 1.0
    iota = np.tile(np.arange(EW, dtype=np.float32)[None, :], (128, W)) \
        .astype(ml_dtypes.bfloat16)
    return xs_c, ys_c, mq, iota


def _core_in_map(images, theta, img_ids):
    xs_c, ys_c, mq, iota = _host_constants()
    imgs = images[img_ids].reshape(-1)
    imgs = np.concatenate([imgs, np.zeros(PAD, np.float32)])
    th = theta[img_ids].reshape(1, BPC * 6)
    return {"images": imgs, "theta": th, "xs_c": xs_c, "ys_c": ys_c,
            "mq_c": mq, "iota_c": iota}


def _get_state(theta):
    import jax
    key = theta.tobytes()
    if _CACHE.get("key") != key:
        acts = _activity(theta)
        perm = _assign(acts)
        devices = jax.devices()[:NCORES]
        runners = []
        for c in range(NCORES):
            th8 = theta[perm[c]]
            acts8 = [acts[m] for m in perm[c]]
            nc = _build_program(th8, acts8)
            runners.append(_Runner(nc, devices[c]))
        _CACHE.clear()
        _CACHE.update({"key": key, "perm": perm, "runners": runners})
    return _CACHE


def kernel(images, theta):
    import jax
    images = np.ascontiguousarray(images, dtype=np.float32)
    theta = np.ascontiguousarray(theta, dtype=np.float32)
    assert images.shape == (B, H, W, C) and theta.shape == (B, 2, 3)
    st = _get_state(theta)
    outs = []
    for c in range(NCORES):
        outs.append(st["runners"][c].fn(
            *st["runners"][c].make_args(
                _core_in_map(images, theta, st["perm"][c]))))
    jax.block_until_ready(outs)
    out = np.empty((B, H, W, C), np.float32)
    for c in range(NCORES):
        res = np.asarray(outs[c][0]).reshape(BPC, H, W, C)
        for k, m in enumerate(st["perm"][c]):
            out[m] = res[k]
    return out


# revision 27
# speedup vs baseline: 1.4360x; 1.3854x over previous
"""Bilinear interpolation (spatial transformer) Trainium2 kernel.

Strategy (data parallel, 8 images per NeuronCore, theta-specialized):

Phase A (per image): build a DRAM table of window slots. Slot (y, k) holds
rows {y, y+1} x cols [5k, 5k+6) x 3 channels in bf16, layout [row][col][ch]
(36 bf16 used of a 128-bf16 / 256B element). 384*77 = 29568 slots per image,
so slot ids fit int16 -- required by dma_gather. Only y-tiles actually
sampled (host-computed from theta) are built.

Phase B (per output third = 128 rows x 384 cols): compute the affine sample
position and the reference's exact floor/clip arithmetic on device; slot id
F = y0*77 + floor(x0/5) (floors via the f32 magic-number trick, 1.5*2^23).
One 256B dma_gather element per output pixel, 1024 indices per instruction
(the SWDGE descriptor-ring carveout limit). The index list is wrapped
[16, n/16] replicated across the 8 GPSIMD core groups; per-q selection
matmuls on the otherwise-idle tensor engine produce that layout directly.
Horizontal selection in the 6-wide window is the stencil
u[b] = wx0*[b==rx] + wx1*[b==rx+1]; vertical uses the reference weights.

Theta specialization: out-of-range pixels are exactly 0 (the reference's
clip arithmetic zero-weights them), and the active region per output row is
an interval, convex over each third. The host computes a conservative
active column range per (image, third); gathers/compute outside it are
skipped statically and zeros are written instead. Because activity differs
per image, each core gets its own specialized program, with images assigned
to cores balancing total active work. Programs are rebuilt if theta changes
(cached on theta bytes), so the kernel stays correct for any input.
"""
import sys

sys.path.insert(0, "/opt/trn_rl_repo")

import numpy as np
import ml_dtypes

H = 384
W = 384
C = 3
B = 64
NCORES = 8
BPC = B // NCORES          # images per core
NT = 3                     # thirds per image
IMG_ELEMS = H * W * C      # 442368
ROW_ELEMS = W * C          # 1152
PAD = 4096                 # padding after the last image (overhang reads)

S = 5                      # slot x-stride (pixels)
EW = 6                     # slot window width (pixels)
NK = 77                    # slots per row (k = floor(x0/5) in 0..76)
NSLOT = H * NK             # 29568 slots per image (< 32768: int16 indices)
ESZ = 128                  # bf16 elems per slot element (256B)

GC = 8                     # output columns per gather (1024 descriptors)
JC = 64                    # output columns per selection chunk
NCHUNK = W // JC           # 6

ABLATE = set()  # {'gather','sel','fold','phaseB'}
NSWQ = 1  # SWDGE queues; gathers round-robin across them

_CACHE = {}


def _activity(theta_all):
    """Host-side conservative activity analysis per image (float64, +/-2px
    margins). Returns per image: thirds{T: (cs_lo, cs_hi) | None} at GC
    granularity, build (list of y-tiles to build), cost (active gathers)."""
    xs = np.linspace(-1.0, 1.0, W)
    ys = np.linspace(-1.0, 1.0, H)
    xn, yn = np.meshgrid(xs, ys)
    out = []
    for m in range(theta_all.shape[0]):
        t = theta_all[m].astype(np.float64)
        x = ((t[0, 0] * xn + t[0, 1] * yn + t[0, 2]) + 1.0) * (W * 0.5)
        y = ((t[1, 0] * xn + t[1, 1] * yn + t[1, 2]) + 1.0) * (H * 0.5)
        act = (x > -2) & (x < W + 1) & (y > -2) & (y < H + 1)
        thirds = {}
        ybounds = []
        cost = 0
        for T in range(NT):
            a = act[128 * T : 128 * T + 128]
            ja = a.any(axis=0)
            if not ja.any():
                thirds[T] = None
                continue
            cs = ja.reshape(W // GC, GC).any(axis=1)
            lo = int(np.argmax(cs))
            hi = (W // GC) - int(np.argmax(cs[::-1]))
            thirds[T] = (lo, hi)
            cost += hi - lo
            # slot rows referenced by gathers = clip(floor(y), 0, 383) over
            # the WHOLE issued-chunk region (inactive pixels there carry
            # clipped slot ids too; their windows are gathered then
            # zero-weighted, so the rows must exist in the table)
            yr = y[128 * T : 128 * T + 128, GC * lo : GC * hi]
            ylo_r = min(H - 1, max(0, int(np.floor(yr.min())) - 2))
            yhi_r = min(H - 1, max(0, int(np.floor(yr.max())) + 2))
            ybounds.append((ylo_r, yhi_r))
        build = set()
        for (ylo, yhi) in ybounds:
            for Ty in range(NT):
                if ylo <= 128 * Ty + 127 and yhi >= 128 * Ty:
                    build.add(Ty)
        out.append({"thirds": thirds, "build": sorted(build), "cost": cost})
    return out


def _group(acts, theta_all):
    """Group images into BPC slots of NCORES images with similar window
    widths (each slot's static schedule runs the max width over its group).
    Returns (perm[c][k]=image, sched[k]={jw:{T: JW|None}, build:[tiles]},
    place[c][k]={T: jlo})."""
    xs = np.linspace(-1.0, 1.0, W)
    ys = np.linspace(-1.0, 1.0, H)
    xn, yn = np.meshgrid(xs, ys)
    nimg = len(acts)
    order = sorted(range(nimg), key=lambda m: -acts[m]["cost"])
    perm = [[None] * BPC for _ in range(NCORES)]
    sched = []
    place = [[{} for _ in range(BPC)] for _ in range(NCORES)]
    for k in range(BPC):
        grp = order[NCORES * k : NCORES * (k + 1)]
        for c, m in enumerate(grp):
            perm[c][k] = m
        jws = {}
        build = set()
        for T in range(NT):
            wmax = 0
            for m in grp:
                r = acts[m]["thirds"][T]
                if r is not None:
                    wmax = max(wmax, GC * (r[1] - r[0]))
            if wmax == 0:
                jws[T] = None
                for c in range(NCORES):
                    place[c][k][T] = 0
                continue
            jw = min(-(-wmax // JC) * JC, W)
            jws[T] = jw
            for c, m in enumerate(grp):
                r = acts[m]["thirds"][T]
                if r is None:
                    place[c][k][T] = 0
                    jlo = 0
                else:
                    lo, hi = GC * r[0], GC * r[1]
                    jlo = max(0, min(lo, W - jw))
                    assert jlo <= lo and jlo + jw >= hi
                    place[c][k][T] = jlo
                # slot rows referenced over the padded window region
                t = theta_all[m].astype(np.float64)
                yv = ((t[1, 0] * xn + t[1, 1] * yn + t[1, 2]) + 1.0) * (H * 0.5)
                yr = yv[128 * T : 128 * T + 128, jlo : jlo + jw]
                ylo_r = min(H - 1, max(0, int(np.floor(yr.min())) - 2))
                yhi_r = min(H - 1, max(0, int(np.floor(yr.max())) + 2))
                for Ty in range(NT):
                    if ylo_r <= 128 * Ty + 127 and yhi_r >= 128 * Ty:
                        build.add(Ty)
        sched.append({"jw": jws, "build": sorted(build)})
    return perm, sched, place


def _build_program(sched):
    """Build the SPMD program from the per-slot static schedule. Per-core
    differences (window positions, images, theta) are inputs, not code."""
    import concourse.bass as bass
    import concourse.bacc as bacc
    import concourse.mybir as mybir
    from concourse import tile

    f32 = mybir.dt.float32
    bf16 = mybir.dt.bfloat16
    i16 = mybir.dt.int16
    i32 = mybir.dt.int32
    OP = mybir.AluOpType

    nc = bacc.Bacc("TRN2", target_bir_lowering=False, debug=False,
                   num_devices=NCORES, num_swdge_queues=NSWQ)

    images = nc.dram_tensor("images", [BPC * IMG_ELEMS + PAD], f32,
                            kind="ExternalInput")
    theta = nc.dram_tensor("theta", [1, BPC * 6], f32, kind="ExternalInput")
    xsw_c = nc.dram_tensor("xsw_c", [BPC * NT * W], f32, kind="ExternalInput")
    ys_c = nc.dram_tensor("ys_c", [128, NT], f32, kind="ExternalInput")
    mq_c = nc.dram_tensor("mq_c", [128, 8 * 128], f32, kind="ExternalInput")
    iota_c = nc.dram_tensor("iota_c", [128, W * EW], bf16, kind="ExternalInput")
    offs_c = nc.dram_tensor("offs_c", [128, BPC * NT], i32, kind="ExternalInput")
    out_d = nc.dram_tensor("out", [BPC * IMG_ELEMS, 1], f32, kind="ExternalOutput")
    tabs = [nc.dram_tensor(f"tab_{m}", [NSLOT, ESZ], bf16) for m in range(BPC)]

    def dram_ap(t, off, layout):
        return bass.AP(t, off, layout)

    # ---------------- phase A: tables + output pre-zero --------------------
    # Own TileContext: the exit barrier orders table writes before gathers
    # and the output pre-zero before the window scatters.
    with tile.TileContext(nc) as tc:
        with (
            tc.tile_pool(name="abuild", bufs=2) as apool,
            tc.tile_pool(name="atb", bufs=1) as tpool,
        ):
            zrow = tpool.tile([128, ROW_ELEMS], f32, tag="zrow")
            nc.vector.memset(zrow[:], 0.0)
            for k in range(BPC):
                for T in range(NT):
                    nc.sync.dma_start(
                        dram_ap(out_d, k * IMG_ELEMS + T * 128 * ROW_ELEMS,
                                [[ROW_ELEMS, 128], [1, ROW_ELEMS]]),
                        zrow[:],
                    )
            tbs = [tpool.tile([128, NK * ESZ], bf16, tag=f"tb{i}",
                              name=f"tb{i}")
                   for i in range(2)]
            nc.vector.memset(tbs[0][:], 0.0)
            nc.vector.memset(tbs[1][:], 0.0)
            nbuilt = 0
            for k in range(BPC):
                base = k * IMG_ELEMS
                for T in sched[k]["build"]:
                    im0 = apool.tile([128, ROW_ELEMS + 24], f32, tag="im0")
                    im1 = apool.tile([128, ROW_ELEMS + 24], f32, tag="im1")
                    off0 = base + T * 128 * ROW_ELEMS
                    nc.sync.dma_start(
                        im0[:],
                        dram_ap(images, off0,
                                [[ROW_ELEMS, 128], [1, ROW_ELEMS + 24]]),
                    )
                    # rows +1; for T==2 partition 127 reads row 384 = next
                    # image / zero pad: those taps are zero-weighted.
                    nc.sync.dma_start(
                        im1[:],
                        dram_ap(images, off0 + ROW_ELEMS,
                                [[ROW_ELEMS, 128], [1, ROW_ELEMS + 24]]),
                    )
                    tb = tbs[nbuilt % 2]
                    nbuilt += 1
                    tb3 = tb[:].rearrange("p (k w) -> p k w", w=ESZ)

                    def win15(t_):
                        a = t_[:, 0 : NK * 15].rearrange(
                            "p (k x) -> p k x", x=15)
                        b_ = t_[:, 15 : 15 + NK * 15].rearrange(
                            "p (k x) -> p k x", x=15)
                        return a, b_

                    a0, b0 = win15(im0)
                    a1, b1 = win15(im1)
                    nc.scalar.copy(tb3[:, :, 0:15], a0)
                    nc.scalar.copy(tb3[:, :, 15:18], b0[:, :, 0:3])
                    nc.vector.tensor_copy(tb3[:, :, 18:33], a1)
                    nc.vector.tensor_copy(tb3[:, :, 33:36], b1[:, :, 0:3])
                    nc.sync.dma_start(
                        dram_ap(tabs[k], T * 128 * NK * ESZ,
                                [[NK * ESZ, 128], [1, NK * ESZ]]),
                        tb[:],
                    )

    # ---------------- phase B: sample + gather + combine + scatter ---------
    with tile.TileContext(nc) as tc:
        with (
            tc.tile_pool(name="consts", bufs=1) as cpool,
            tc.tile_pool(name="work", bufs=2) as wpool,
            tc.tile_pool(name="idx", bufs=2) as ipool,
            tc.tile_pool(name="sel", bufs=2) as spool,
            tc.tile_pool(name="ps", bufs=2, space="PSUM") as ppool,
        ):
            ys_t = cpool.tile([128, NT], f32, tag="ys")
            nc.sync.dma_start(ys_t[:], ys_c[:])
            mq_t = cpool.tile([128, 8 * 128], f32, tag="mq")
            nc.sync.dma_start(mq_t[:], mq_c[:])
            iota_t = cpool.tile([128, W * EW], bf16, tag="iota")
            nc.sync.dma_start(iota_t[:], iota_c[:])
            offs_t = cpool.tile([128, BPC * NT], i32, tag="offs")
            nc.sync.dma_start(offs_t[:], offs_c[:])
            th_row = cpool.tile([1, BPC * 6], f32, tag="throw")
            nc.sync.dma_start(th_row[:], theta[:])
            th = cpool.tile([128, BPC * 6], f32, tag="th")
            nc.gpsimd.partition_broadcast(th[:], th_row[:])
            gts = [cpool.tile([128, JC * ESZ], bf16, tag=f"g{i}",
                              name=f"g{i}")
                   for i in range(2)]
            nc.vector.memset(gts[0][:], 0.0)
            nc.vector.memset(gts[1][:], 0.0)

            i6b = iota_t[:].rearrange("p (j b) -> p j b", b=EW)
            MAGIC = 12582912.0  # 1.5*2^23: integer-ULP zone for |v| < 2^22
            ngat = 0
            gq = 0

            for k in range(BPC):
                t00 = th[:, k * 6 + 0 : k * 6 + 1]
                t01 = th[:, k * 6 + 1 : k * 6 + 2]
                t02 = th[:, k * 6 + 2 : k * 6 + 3]
                t10 = th[:, k * 6 + 3 : k * 6 + 4]
                t11 = th[:, k * 6 + 4 : k * 6 + 5]
                t12 = th[:, k * 6 + 5 : k * 6 + 6]
                for T in range(NT):
                    jw = sched[k]["jw"][T]
                    if jw is None:
                        continue
                    jsl = slice(0, jw)
                    col = k * NT + T

                    # window xn values, replicated across partitions by a
                    # stride-0 DMA read of this core's xsw row
                    xw = wpool.tile([128, W], f32, tag="xw")
                    nc.sync.dma_start(
                        xw[:, jsl],
                        dram_ap(xsw_c, col * W, [[0, 128], [1, jw]]),
                    )

                    ysT = ys_t[:, T : T + 1]
                    sc = wpool.tile([128, 8], f32, tag="sc")
                    nc.vector.tensor_scalar(sc[:, 0:1], t00, float(W) * 0.5, None, OP.mult)
                    nc.vector.tensor_scalar(sc[:, 1:2], ysT, t01, t02, OP.mult, OP.add)
                    nc.vector.tensor_scalar(sc[:, 2:3], sc[:, 1:2], 1.0, float(W) * 0.5, OP.add, OP.mult)
                    nc.vector.tensor_scalar(sc[:, 3:4], t10, float(H) * 0.5, None, OP.mult)
                    nc.vector.tensor_scalar(sc[:, 4:5], ysT, t11, t12, OP.mult, OP.add)
                    nc.vector.tensor_scalar(sc[:, 5:6], sc[:, 4:5], 1.0, float(H) * 0.5, OP.add, OP.mult)

                    x = wpool.tile([128, W], f32, tag="x")
                    y = wpool.tile([128, W], f32, tag="y")
                    nc.vector.tensor_scalar(x[:, jsl], xw[:, jsl], sc[:, 0:1],
                                            sc[:, 2:3], OP.mult, OP.add)
                    nc.vector.tensor_scalar(y[:, jsl], xw[:, jsl], sc[:, 3:4],
                                            sc[:, 5:6], OP.mult, OP.add)

                    x0f = wpool.tile([128, W], f32, tag="x0f")
                    y0f = wpool.tile([128, W], f32, tag="y0f")
                    nc.vector.tensor_scalar(x0f[:, jsl], x[:, jsl], -0.5, MAGIC, OP.add, OP.add)
                    nc.vector.tensor_scalar(x0f[:, jsl], x0f[:, jsl], -MAGIC, None, OP.add)
                    nc.vector.tensor_scalar(y0f[:, jsl], y[:, jsl], -0.5, MAGIC, OP.add, OP.add)
                    nc.vector.tensor_scalar(y0f[:, jsl], y0f[:, jsl], -MAGIC, None, OP.add)

                    x0c = wpool.tile([128, W], f32, tag="x0c")
                    x1c = wpool.tile([128, W], f32, tag="x1c")
                    y0c = wpool.tile([128, W], f32, tag="y0c")
                    y1c = wpool.tile([128, W], f32, tag="y1c")
                    nc.vector.tensor_scalar(x0c[:, jsl], x0f[:, jsl], 0.0, float(W - 1), OP.max, OP.min)
                    nc.vector.tensor_scalar(x1c[:, jsl], x0f[:, jsl], -1.0, 1.0, OP.max, OP.add)
                    nc.vector.tensor_scalar(x1c[:, jsl], x1c[:, jsl], float(W - 1), None, OP.min)
                    nc.vector.tensor_scalar(y0c[:, jsl], y0f[:, jsl], 0.0, float(H - 1), OP.max, OP.min)
                    nc.vector.tensor_scalar(y1c[:, jsl], y0f[:, jsl], -1.0, 1.0, OP.max, OP.add)
                    nc.vector.tensor_scalar(y1c[:, jsl], y1c[:, jsl], float(H - 1), None, OP.min)

                    xc = wpool.tile([128, W], f32, tag="xc")
                    yc = wpool.tile([128, W], f32, tag="yc")
                    nc.vector.tensor_scalar(xc[:, jsl], x[:, jsl], 0.0, float(W - 1), OP.max, OP.min)
                    nc.vector.tensor_scalar(yc[:, jsl], y[:, jsl], 0.0, float(H - 1), OP.max, OP.min)

                    wx0 = wpool.tile([128, W], f32, tag="wx0")
                    wx1 = wpool.tile([128, W], f32, tag="wx1")
                    wy0 = wpool.tile([128, W], f32, tag="wy0")
                    wy1 = wpool.tile([128, W], f32, tag="wy1")
                    nc.vector.tensor_tensor(wx0[:, jsl], x1c[:, jsl], xc[:, jsl], OP.subtract)
                    nc.vector.tensor_tensor(wx1[:, jsl], xc[:, jsl], x0c[:, jsl], OP.subtract)
                    nc.vector.tensor_tensor(wy0[:, jsl], y1c[:, jsl], yc[:, jsl], OP.subtract)
                    nc.vector.tensor_tensor(wy1[:, jsl], yc[:, jsl], y0c[:, jsl], OP.subtract)

                    # k = floor(x0c/5): v=0.2f*x0c is within 1e-5 of the grid
                    # {m+0.2i}, so bias -0.49 keeps rne(v-0.49)==m tie-free
                    kf = wpool.tile([128, W], f32, tag="kf")
                    nc.vector.tensor_scalar(
                        kf[:, jsl], x0c[:, jsl], float(np.float32(1.0 / S)),
                        -0.49, OP.mult, OP.add)
                    nc.vector.tensor_scalar(kf[:, jsl], kf[:, jsl], MAGIC, None, OP.add)
                    nc.vector.tensor_scalar(kf[:, jsl], kf[:, jsl], -MAGIC, None, OP.add)
                    rx = wpool.tile([128, W], f32, tag="rx")
                    rxp = wpool.tile([128, W], f32, tag="rxp")
                    nc.vector.scalar_tensor_tensor(
                        rx[:, jsl], kf[:, jsl], float(-S), x0c[:, jsl],
                        op0=OP.mult, op1=OP.add)
                    nc.vector.tensor_scalar(rxp[:, jsl], rx[:, jsl], 1.0, None, OP.add)
                    fsl = wpool.tile([128, W], f32, tag="fsl")
                    nc.vector.scalar_tensor_tensor(
                        fsl[:, jsl], y0c[:, jsl], float(NK), kf[:, jsl],
                        op0=OP.mult, op1=OP.add)

                    rx16 = wpool.tile([128, W], bf16, tag="rx16")
                    rxp16 = wpool.tile([128, W], bf16, tag="rxp16")
                    wx016 = wpool.tile([128, W], bf16, tag="wx016")
                    wx116 = wpool.tile([128, W], bf16, tag="wx116")
                    wy016 = wpool.tile([128, W], bf16, tag="wy016")
                    wy116 = wpool.tile([128, W], bf16, tag="wy116")
                    nc.scalar.copy(rx16[:, jsl], rx[:, jsl])
                    nc.scalar.copy(rxp16[:, jsl], rxp[:, jsl])
                    nc.scalar.copy(wx016[:, jsl], wx0[:, jsl])
                    nc.scalar.copy(wx116[:, jsl], wx1[:, jsl])
                    nc.scalar.copy(wy016[:, jsl], wy0[:, jsl])
                    nc.scalar.copy(wy116[:, jsl], wy1[:, jsl])

                    def jb(t_):
                        return t_[:, jsl].rearrange("p (j o) -> p j o", o=1) \
                                         .to_broadcast([128, jw, EW])

                    u = wpool.tile([128, W * EW], bf16, tag="u")
                    ut = wpool.tile([128, W * EW], bf16, tag="ut")
                    u3 = u[:].rearrange("p (j b) -> p j b", b=EW)[:, jsl, :]
                    ut3 = ut[:].rearrange("p (j b) -> p j b", b=EW)[:, jsl, :]
                    i6s = i6b[:, jsl, :]
                    nc.vector.tensor_tensor(u3, i6s, jb(rx16), OP.is_equal)
                    nc.vector.tensor_tensor(u3, u3, jb(wx016), OP.mult)
                    nc.vector.tensor_tensor(ut3, i6s, jb(rxp16), OP.is_equal)
                    nc.vector.tensor_tensor(ut3, ut3, jb(wx116), OP.mult)
                    nc.vector.tensor_tensor(u3, u3, ut3, OP.add)

                    # wrapped+replicated int16 indices:
                    # idx16[P, 8j+q] = fsl[16q + P%16, j]
                    idx16 = ipool.tile([128, W * 8], i16, tag="idx16")
                    idxq = idx16[:].rearrange("p (j q) -> p j q", q=8)
                    for q in range(8):
                        ps = ppool.tile([128, W], f32, tag="ps")
                        nc.tensor.matmul(
                            out=ps[:, 0:jw],
                            lhsT=mq_t[:, q * 128 : (q + 1) * 128],
                            rhs=fsl[:, jsl],
                            start=True,
                            stop=True,
                        )
                        nc.vector.tensor_copy(idxq[:, jsl, q], ps[:, 0:jw])

                    acc = wpool.tile([128, ROW_ELEMS], f32, tag="acc")
                    for cc in range(jw // JC):
                        g = gts[ngat % 2]
                        ngat += 1
                        g3 = g[:].rearrange("p (j w) -> p j w", w=ESZ)
                        for s_ in range(8):
                            cs = cc * 8 + s_
                            nc.gpsimd.dma_gather(
                                g[:, GC * ESZ * s_ : GC * ESZ * (s_ + 1)]
                                    .rearrange("p (b e) -> p b e", e=ESZ),
                                tabs[k][:],
                                idx16[:, GC * 8 * cs : GC * 8 * (cs + 1)],
                                GC * 128,
                                GC * 128,
                                ESZ,
                                queue_num=gq % NSWQ,
                            )
                            gq += 1
                        js = slice(JC * cc, JC * (cc + 1))

                        def cb(t_):
                            return t_[:, js].rearrange("p (j o) -> p j o", o=1) \
                                            .to_broadcast([128, JC, C])

                        u4 = u[:].rearrange("p (j b) -> p j b", b=EW)
                        a0 = spool.tile([128, JC * C], bf16, tag="a0")
                        a1 = spool.tile([128, JC * C], bf16, tag="a1")
                        tt_ = spool.tile([128, JC * C], bf16, tag="tt")
                        t3f = spool.tile([128, JC * C], f32, tag="t3f")
                        a03 = a0[:].rearrange("p (j c) -> p j c", c=C)
                        a13 = a1[:].rearrange("p (j c) -> p j c", c=C)
                        t3 = tt_[:].rearrange("p (j c) -> p j c", c=C)
                        t3f3 = t3f[:].rearrange("p (j c) -> p j c", c=C)

                        def ub(b):
                            return u4[:, js, b : b + 1].to_broadcast([128, JC, C])

                        for r, ar in ((0, a03), (1, a13)):
                            base_w = r * EW * C
                            nc.vector.tensor_tensor(
                                ar, g3[:, :, base_w : base_w + C], ub(0), OP.mult)
                            for b_ in range(1, EW):
                                nc.vector.tensor_tensor(
                                    t3, g3[:, :, base_w + b_ * C : base_w + (b_ + 1) * C],
                                    ub(b_), OP.mult)
                                nc.vector.tensor_tensor(ar, ar, t3, OP.add)

                        a3 = acc[:, JC * C * cc : JC * C * (cc + 1)] \
                            .rearrange("p (j c) -> p j c", c=C)
                        nc.vector.tensor_tensor(a3, a03, cb(wy016), OP.mult)
                        nc.vector.tensor_tensor(t3f3, a13, cb(wy116), OP.mult)
                        nc.vector.tensor_tensor(a3, a3, t3f3, OP.add)

                    # scatter the window to its per-core column position
                    nc.gpsimd.indirect_dma_start(
                        out=out_d[:],
                        out_offset=bass.IndirectOffsetOnAxis(
                            ap=offs_t[:, col : col + 1], axis=0),
                        in_=acc[:, 0 : jw * C],
                        in_offset=None,
                    )

    nc.compile()
    return nc


class _Runner:
    def __init__(self, nc, n_cores):
        import jax
        from jax.sharding import Mesh, PartitionSpec
        from jax.experimental.shard_map import shard_map
        import concourse.mybir as mybir
        from concourse.bass2jax import (
            _bass_exec_p, partition_id_tensor, install_neuronx_cc_hook,
        )

        install_neuronx_cc_hook()
        self.jax = jax
        self.n_cores = n_cores
        partition_name = nc.partition_id_tensor.name if nc.partition_id_tensor else None
        in_names, out_names, out_avals, zero_outs = [], [], [], []
        for alloc in nc.m.functions[0].allocations:
            if not isinstance(alloc, mybir.MemoryLocationSet):
                continue
            name = alloc.memorylocations[0].name
            if alloc.kind == "ExternalInput":
                if name != partition_name:
                    in_names.append(name)
            elif alloc.kind == "ExternalOutput":
                shape = tuple(alloc.tensor_shape)
                dtype = mybir.dt.np(alloc.dtype)
                out_avals.append(jax.core.ShapedArray(shape, dtype))
                out_names.append(name)
                zero_outs.append(np.zeros(shape, dtype))
        self.in_names = list(in_names)
        self.out_names = out_names
        self.zero_outs = zero_outs
        n_params = len(in_names)
        n_outs = len(out_names)
        all_in_names = in_names + out_names
        if partition_name is not None:
            all_in_names.append(partition_name)

        def _body(*args):
            operands = list(args)
            if partition_name is not None:
                operands.append(partition_id_tensor())
            outs = _bass_exec_p.bind(
                *operands,
                out_avals=tuple(out_avals),
                in_names=tuple(all_in_names),
                out_names=tuple(out_names),
                lowering_input_output_aliases=(),
                sim_require_finite=False,
                sim_require_nnan=False,
                nc=nc,
            )
            return tuple(outs)

        devices = jax.devices()[:n_cores]
        self.mesh = Mesh(np.asarray(devices), ("core",))
        in_specs = (PartitionSpec("core"),) * (n_params + n_outs)
        out_specs = (PartitionSpec("core"),) * n_outs
        self.fn = jax.jit(
            shard_map(_body, mesh=self.mesh, in_specs=in_specs,
                      out_specs=out_specs, check_rep=False),
            keep_unused=True,
        )

    def make_args(self, in_maps):
        from jax.sharding import NamedSharding, PartitionSpec
        sharding = NamedSharding(self.mesh, PartitionSpec("core"))
        concat = [
            np.concatenate([np.asarray(m[name]) for m in in_maps], axis=0)
            for name in self.in_names
        ]
        concat += [
            np.zeros((self.n_cores * z.shape[0], *z.shape[1:]), z.dtype)
            for z in self.zero_outs
        ]
        return [self.jax.device_put(a, sharding) for a in concat]

    def run(self, in_maps):
        outs = self.fn(*self.make_args(in_maps))
        self.jax.block_until_ready(outs)
        res = []
        for c in range(self.n_cores):
            d = {}
            for i, name in enumerate(self.out_names):
                a = np.asarray(outs[i])
                per_core = (self.n_cores, a.shape[0] // self.n_cores) + a.shape[1:]
                d[name] = a.reshape(per_core)[c]
            res.append(d)
        return res


def _host_constants():
    import jax.numpy as jnp

    xs = np.asarray(jnp.linspace(-1.0, 1.0, W, dtype=jnp.float32))
    ys = np.asarray(jnp.linspace(-1.0, 1.0, H, dtype=jnp.float32))
    ys_c = ys.reshape(NT, 128).T.copy().astype(np.float32)
    mq = np.zeros((128, 8 * 128), np.float32)
    for q in range(8):
        for P in range(128):
            mq[16 * q + (P % 16), q * 128 + P] = 1.0
    iota = np.tile(np.arange(EW, dtype=np.float32)[None, :], (128, W)) \
        .astype(ml_dtypes.bfloat16)
    return xs, ys_c, mq, iota


def _core_in_map(images, theta, img_ids, placek):
    xs, ys_c, mq, iota = _host_constants()
    imgs = images[img_ids].reshape(-1)
    imgs = np.concatenate([imgs, np.zeros(PAD, np.float32)])
    th = theta[img_ids].reshape(1, BPC * 6)
    xsw = np.zeros((BPC * NT * W,), np.float32)
    offs = np.zeros((128, BPC * NT), np.int32)
    p = np.arange(128)
    for k in range(BPC):
        for T in range(NT):
            jlo = placek[k][T]
            cols = np.minimum(jlo + np.arange(W), W - 1)
            xsw[(k * NT + T) * W : (k * NT + T + 1) * W] = xs[cols]
            offs[:, k * NT + T] = (k * IMG_ELEMS + (128 * T + p) * ROW_ELEMS
                                   + jlo * C)
    return {"images": imgs, "theta": th, "xsw_c": xsw, "ys_c": ys_c,
            "mq_c": mq, "iota_c": iota, "offs_c": offs}


def _get_state(theta):
    key = theta.tobytes()
    if _CACHE.get("key") != key:
        acts = _activity(theta)
        perm, sched, place = _group(acts, theta)
        nc = _build_program(sched)
        runner = _Runner(nc, NCORES)
        _CACHE.clear()
        _CACHE.update({"key": key, "perm": perm, "sched": sched,
                       "place": place, "runner": runner})
    return _CACHE


def _in_maps(images, theta, st):
    return [
        _core_in_map(images, theta, st["perm"][c], st["place"][c])
        for c in range(NCORES)
    ]


def kernel(images, theta):
    images = np.ascontiguousarray(images, dtype=np.float32)
    theta = np.ascontiguousarray(theta, dtype=np.float32)
    assert images.shape == (B, H, W, C) and theta.shape == (B, 2, 3)
    st = _get_state(theta)
    res = st["runner"].run(_in_maps(images, theta, st))
    out = np.empty((B, H, W, C), np.float32)
    for c in range(NCORES):
        full = res[c]["out"].reshape(BPC, H, W, C)
        for k, m in enumerate(st["perm"][c]):
            out[m] = full[k]
    return out
